# revision 37
# baseline (speedup 1.0000x reference)
"""KMeans vq_codebook step on 8 NeuronCores (Trainium2, Bass/Tile) — v3.

Data-parallel over N: each core gets an x shard [8192, 512]; centers
replicated. Host pre-packs everything so the device hot loop is minimal:

  host:  x -> fp8(e4m3) in DoubleRow weights layout [j, t, h, e, p]
         centers -> 2*c fp8 DoubleRow moving layout [h, j, e, k]
         -c2 -> two fp8 digits (rank-1 DoubleRow seed rows, |err| <= 2)
         y -> one-hot fp8 [p, (pair, i, cls)]
  device per 128-point tile:
         seed  = ones x (-c2 digits) rank-1 DR matmul  (PSUM = -c2)
         s    += 2x@c.T via 4 fp8 DoubleRow matmuls    (PSUM f32)
         m     = DVE tensor_reduce max over PSUM -> m8[:, t]
         mask  = ACT Sign(m - s) in {0 at max, +1 else} (fp8)
         hist += oht^T @ mask via fp8 DoubleRow matmuls (tile pairs)
  host:  loss = sum(x^2) - sum(m8)
         counts[c,k] = bincount(y)[c] - hist[c,k]; acc = sum max-label.
"""
import os
import sys

sys.path.insert(0, "/opt/trn_rl_repo")

import ml_dtypes
import numpy as np

import concourse.bass as bass
import concourse.mybir as mybir
from concourse import bacc
from concourse.bass import ds, ts
from concourse.bass_utils import run_bass_kernel_spmd
from concourse.tile import TileContext

dt = mybir.dt
F32 = dt.float32
BF16 = dt.bfloat16
F8 = dt.float8e4
U8 = dt.uint8
AF = mybir.ActivationFunctionType
ALU = mybir.AluOpType
PM = mybir.MatmulPerfMode

N, D, K, NCLS, NCORES = 65536, 512, 1024, 10, 8
NSH = N // NCORES          # 8192 points per core
PT = NSH // 128            # 64 point-tiles per core
NPAIR = PT // 2            # 32 tile pairs (DoubleRow histogram)
FP8 = ml_dtypes.float8_e4m3  # TRN flavor (max normal 240)
# seed rows hold -c2/4 (|c2| ~ 512 exceeds fp8 range); lhsT rows are 4.0
SEED_SCALE = 4.0
SEED_F8 = int(np.array(SEED_SCALE, FP8).view(np.uint8))

USE_GPSIMD_DMA = False     # odd-tile x loads on the GpSimd DMA queue


def _build(stage=4):
    # bisection stages: 0 loads+warmup; 1 +DR GEMM; 2 +reduce; 3 +mask; 4 full
    nc = bacc.Bacc(None, target_bir_lowering=False, debug=False)
    x_in = nc.dram_tensor("xp", [128, PT * D], U8, kind="ExternalInput")
    cpk_in = nc.dram_tensor("cpk", [2, 128, 2, K], U8, kind="ExternalInput")
    sdr_in = nc.dram_tensor("sdr", [1, 2 * K], U8, kind="ExternalInput")
    oht_in = nc.dram_tensor("oht", [128, NPAIR * 32], U8, kind="ExternalInput")
    counts_out = nc.dram_tensor("counts", [16, K], F32, kind="ExternalOutput")
    m_out = nc.dram_tensor("mrow", [128, PT], F32, kind="ExternalOutput")

    with TileContext(nc) as tc:
        with (
            tc.tile_pool(name="persist", bufs=1) as pp,
            tc.tile_pool(name="work", bufs=3) as wp,
            tc.tile_pool(name="psA", bufs=2, space="PSUM") as psA,
            tc.tile_pool(name="psW", bufs=1, space="PSUM") as psW,
            tc.tile_pool(name="psH", bufs=1, space="PSUM") as psH,
        ):
            # ---- prep: replicated constants in one DMA each
            cpk = pp.tile([128, 2, 2, K], U8)          # [j, h, e, k] = fp8 2*c
            nc.sync.dma_start(out=cpk[:], in_=cpk_in[:, :, :, :].rearrange(
                "h j e k -> j h e k"))
            sdr = pp.tile([1, 2 * K], U8)              # [-c2 digit rows e, k]
            nc.sync.dma_start(out=sdr[:], in_=sdr_in[:, :])
            ohtt = pp.tile([128, NPAIR * 32], U8)      # [p, (pair i c)]
            nc.sync.dma_start(out=ohtt[:], in_=oht_in[:, :])
            ones1 = pp.tile([1, 2, 128], U8)           # rank-1 DR lhsT of 4.0s
            nc.vector.memset(ones1[:], SEED_F8)
            m8 = pp.tile([128, PT], F32)               # rowmax of s per tile

            # ---- PE warmup: open the clock gate before the fp8 stream
            wt_f = pp.tile([128, 128], F32)
            nc.vector.memset(wt_f[:], 0.0)
            wt = wt_f[:].bitcast(BF16)[:, 0:128]
            wps = psW.tile([128, 512], F32, tag="w")
            for _ in range(16):
                nc.tensor.matmul(wps[:, 0:128], wt, wt, start=True, stop=True,
                                 skip_group_check=True)

            hist = None
            if stage >= 4:
                hist = psH.tile([16, K], F32, tag="hist")

            # ---- main loop: 32 tile-pairs
            for pr in range(NPAIR):
                mask2 = wp.tile([128, 2, K], F8, tag="mask")
                for i in range(2):
                    t = 2 * pr + i
                    xq = wp.tile([128, D], U8, tag="xq")
                    dma_eng = nc.sync if (t % 2 == 0 or not USE_GPSIMD_DMA) \
                        else nc.gpsimd
                    dma_eng.dma_start(out=xq[:], in_=x_in[:, ds(t * D, D)])
                    if stage < 1:
                        nc.vector.tensor_reduce(
                            m8[:, t:t + 1], xq[:, 0:4].bitcast(F32),
                            axis=mybir.AxisListType.X, op=ALU.add)
                        continue
                    ps = psA.tile([128, K], F32, tag="ps")
                    for kh in range(2):
                        # rank-1 seed: PSUM = -c2 (two fp8 digit rows)
                        nc.tensor.matmul(
                            ps[:, ds(kh * 512, 512)], ones1[:].bitcast(F8),
                            sdr[:].bitcast(F8).rearrange(
                                "o (e k) -> o e k", e=2)[:, :, ds(kh * 512, 512)],
                            start=True, stop=False, perf_mode=PM.DoubleRow,
                            skip_group_check=True)
                    for h in range(2):
                        lhsT = xq[:, ds(h * 256, 256)].bitcast(F8).rearrange(
                            "j (e p) -> j e p", e=2)
                        for kh in range(2):
                            rhs = cpk[:, h, :, ds(kh * 512, 512)].bitcast(F8)
                            nc.tensor.matmul(ps[:, ds(kh * 512, 512)], lhsT, rhs,
                                             start=False, stop=(h == 1),
                                             perf_mode=PM.DoubleRow,
                                             skip_group_check=True)
                    if stage < 2:
                        nc.vector.tensor_reduce(
                            m8[:, t:t + 1], ps[:, 0:8],
                            axis=mybir.AxisListType.X, op=ALU.add)
                        continue
                    nc.vector.tensor_reduce(m8[:, t:t + 1], ps[:],
                                            axis=mybir.AxisListType.X,
                                            op=ALU.max)
                    if stage < 3:
                        continue
                    # sign(m - s) in {0 at max, +1 else}
                    nc.scalar.activation(mask2[:, i, :], ps[:], AF.Sign,
                                         bias=m8[:, t:t + 1], scale=-1.0)
                if stage < 4:
                    continue
                oh2 = ohtt[:, ds(pr * 32, 32)].bitcast(F8).rearrange(
                    "p (i c) -> p i c", i=2)
                for kh in range(2):
                    nc.tensor.matmul(hist[:, ds(kh * 512, 512)], oh2,
                                     mask2[:, :, ds(kh * 512, 512)],
                                     start=(pr == 0), stop=(pr == NPAIR - 1),
                                     perf_mode=PM.DoubleRow,
                                     skip_group_check=True)

            # ---- tail: outputs
            nc.sync.dma_start(out=m_out[:], in_=m8[:])
            csb = pp.tile([16, K], F32)
            if stage >= 4:
                nc.scalar.copy(csb[:], hist[:])
            else:
                nc.vector.memset(csb[:], 0.0)
            nc.sync.dma_start(out=counts_out[:], in_=csb[:])

    nc.finalize()
    return nc


_NC_CACHE: dict = {}


def _get_nc(stage=4):
    if stage not in _NC_CACHE:
        _NC_CACHE[stage] = _build(stage)
    return _NC_CACHE[stage]


def _pack_inputs(x, centers, y):
    x_f8 = x.astype(FP8)
    # per-core [j, t, h, e, p] with d = 256h + 2j + e
    xq = x_f8.reshape(NCORES, PT, 128, 2, 128, 2).transpose(0, 4, 1, 3, 5, 2)
    xq = np.ascontiguousarray(xq).reshape(NCORES, 128, PT * D).view(np.uint8)
    c_f8 = (2.0 * centers).astype(FP8)                  # [K, 512]
    # [h, j, e, k] with d = 256h + 2j + e
    c_pack = np.ascontiguousarray(
        c_f8.reshape(K, 2, 128, 2).transpose(1, 2, 3, 0)).view(np.uint8)
    c2 = np.einsum("kd,kd->k", centers.astype(np.float64),
                   centers.astype(np.float64)).astype(np.float32)
    # -c2/SEED_SCALE as two fp8 digits: [1, (e, k)]
    tgt = -c2 / SEED_SCALE
    q1 = tgt.astype(FP8)
    q2 = (tgt - q1.astype(np.float32)).astype(FP8)
    sdr = np.ascontiguousarray(
        np.stack([q1, q2], axis=0).reshape(1, 2 * K)).view(np.uint8)
    # one-hot labels, fp8 {0,1}: [core, p, (pair, i, c)]
    oht = np.zeros((N, 16), FP8)
    oht[np.arange(N), y] = FP8(1.0)
    oht = oht.reshape(NCORES, NPAIR, 2, 128, 16).transpose(0, 3, 1, 2, 4)
    oht = np.ascontiguousarray(oht.reshape(NCORES, 128, NPAIR * 32)).view(np.uint8)
    return xq, c_pack, sdr, oht


def kernel(x, centers, y, _trace=False, _use_f32r=True):
    x = np.ascontiguousarray(np.asarray(x, dtype=np.float32))
    centers = np.ascontiguousarray(np.asarray(centers, dtype=np.float32))
    y = np.ascontiguousarray(np.asarray(y, dtype=np.int32))
    nc = _get_nc(int(os.environ.get("KSTAGE", "4")))
    xq, c_pack, sdr, oht = _pack_inputs(x, centers, y)
    in_maps = [
        {"xp": xq[c], "cpk": c_pack, "sdr": sdr, "oht": oht[c]}
        for c in range(NCORES)
    ]
    res = run_bass_kernel_spmd(nc, in_maps, core_ids=list(range(NCORES)),
                               trace=_trace)

    # ---- host finalization
    x64 = x.astype(np.float64)
    loss = np.einsum("nd,nd->", x64, x64)
    counts = np.zeros((16, K), np.float64)
    for c, r in enumerate(res.results):
        loss -= r["mrow"].astype(np.float64).sum()
        # mask is {+1 non-max, 0 at max}: device hist = bincount - counts
        ysh = y[c * NSH:(c + 1) * NSH]
        b = np.bincount(ysh, minlength=16).astype(np.float64)
        counts += b[:16, None] - r["counts"].astype(np.float64)
    correct = counts[:NCLS].max(axis=0).sum()
    acc = np.float32(correct / N)
    out = (np.float32(loss), acc)
    if _trace:
        return out, res
    return out
